# revision 60
# baseline (speedup 1.0000x reference)
"""MoE post-processing MLP kernel for Trainium2 (8 NeuronCores).

Strategy: expert-parallel sharding. Each core is assigned one chunk of
samples routed to a single expert (K=8 experts ~= 8 cores for uniform
routing). The host gathers/permutes samples by expert, computes the 36
posenc sin features (np.sin is cheap host-side and more precise than
the fp16 device path), and the device runs a dense 3-layer MLP in fp16
(fp32 PSUM accumulation):

  h0 = relu(W0a^T@xa + W0s^T@s36 + b0)   xa = [feat,pos,view] 38 rows
  h1 = relu(W1^T@h0 + b1)                s36 = posenc sins, 36 rows
  y  = W2^T@h1 + b2

Device layout: pair-packed (2 samples per column; weights duplicated
block-diagonally so the full 128-partition contract dim is used).
COLS = 8*512 + 128 = 4224 columns = 8448 samples per core, sized to
the actual max per-expert count (8336) instead of a generic bound.

Processing unit is a 1024-col group (two 512-col PSUM tiles), with the
PE issue stream SOFTWARE-PIPELINED so every matmul's producers ran at
least one group earlier (no within-group PE->Scalar->PE round trips):
  iter i issues:  W0a/W0s(seq[i+1]) x4 | W1(seq[i]) x2 | W2(seq[i-1]) x2
where seq = [0, tail, 1, 2, 3]: the 128-col tail group runs second so
its serial mini-chain hides inside the early pipeline and the kernel
drains on a full group.
W2(t0) targets PSUM partitions 0:64 and W2(t1) partitions 64:128 of
ONE [128,512] bank via PE tile_position=(0,64) (the second tile of the
pair runs in the free half of the PE array at near-zero cost), so a
single full-width tensor_scalar emits y for both tiles (the ACT/DVE
fixed ~350-cycle per-op overhead makes many small ops the enemy).
Element-wise: relu0 per 1024 on Scalar (one activation Relu + b0 per
group), relu1 per 512 on Vector (add-b1/max-0, separate h1t tiles so
each W2 half starts as soon as its half is ready), y_pair alternating
Scalar/Vector per group; Pool cannot read PSUM on TRN2.
PSUM: h0p 1024x2bufs (4 banks) + h1p 512x2bufs (2) + yp 512x2bufs (2).

Input chunks are SEPARATE SBUF tiles (fa0/fa1/.., fs0/..): the Tile
framework tracks dependencies per tile, so a single [76, COLS] tile
would make the first matmul wait on the LAST bulk DMA chunk (observed
8us PE stall).  Rings: Pool software ring streams the fa chunks,
Scalar hwdge the fs chunks (per-queue transfers serialize at
~40-80 GB/s, so the two tensors ride separate queues in consumption
order); SP carries wall/bias and all y outputs.

The PE clock governor samples duty EARLY in the kernel (~8-14us) and
the decision sticks: any multi-us PE idle in that window locks the
kernel at 1.2 GHz (427ns/512-row matmul) while 100% duty yields
2.4 GHz (217ns).  Input DMA signals per-entry (first chunk ready ~10.5-12us), but the
combined queue supply (~150 GB/s) only just sustains full-rate
consumption, so the stream starts once ~2 groups of backlog exist: a
16-matmul zero-weight warmup chain (accumulating zeros into g0's live
h0p group, which survives DCE) bridges engine start to that point, and
one zero-weight filler matmul inside the first two lookahead groups'
open h0p accumulation absorbs supply jitter without letting the PE
idle; fillers past the ~16us governor window are pure cost.  The final
group's output splits into parallel SP + Scalar entries (Scalar's
compute is done by then; mid-stream Scalar dispatches steal from the
relu queue, and outputs must never ride the Pool software ring, whose
end-of-ring DRAIN would trail the last transfer by ~2us).
"""

import numpy as np

K = 8
WID = 64
D = 32
NT = 512            # full-tile matmul moving dim (one fp32 PSUM bank)
NFULL = 8           # full tiles (1024 samples each, pair-packed)
TNT = 128           # tail-tile moving dim (256 samples)
C = NFULL * 2 * NT + 2 * TNT     # 8448 samples per core-chunk
COLS = NFULL * NT + TNT          # 4224 device columns
NGRP = NFULL // 2   # 4 full groups of 1024 cols; group NGRP = tail
YCOLS = NGRP * NT + TNT          # 2176 output dram columns

RA = 38             # xa rows: feat 32 + pos 3 + view 3
RS = 36             # s36 rows

# input chunk boundaries (512-aligned; separate SBUF tiles per chunk).
# DMA packet length = chunk width * 2B per partition row; wider chunks
# amortize the ~20-45ns per-packet queue issue cost, but the first
# chunk must stay small so the pipeline can start early.  Queue rates
# measured: Pool swdge ~81 GB/s, Scalar hwdge ~42, SP hwdge ~30; fa
# rides Pool alone, fs alternates Scalar/SP so three queues stream in
# parallel, each in consumption order.
FA_CHUNKS = [(0, 512), (512, 1024), (4096, COLS), (1024, 2048),
             (2048, 3072), (3072, 4096)]
FS_CHUNKS = [(0, 512), (512, 1024), (4096, COLS), (1024, 2048),
             (2048, 3072), (3072, 4096)]
FS_RINGS = ["scalar"] * 6

# W0 row indices (DIN=74 layout: feat 0:32, posenc(pos,2) 32:47,
# posenc(view,4) 47:74) for the identity part and the sin part.
_W0A_ROWS = list(range(32)) + [32, 33, 34] + [47, 48, 49]
_W0S_ROWS = (list(range(35, 41)) + list(range(50, 62))
             + list(range(41, 47)) + list(range(62, 74)))

_PREP = None  # compiled Bass program, built once per process
_LAST_IN_MAPS = None  # stashed for external profiling harnesses


def _build_program():
    import concourse.bacc as bacc
    import concourse.mybir as mybir
    from concourse.tile import TileContext

    F32, F16 = mybir.dt.float32, mybir.dt.float16
    AF = mybir.ActivationFunctionType
    ALU = mybir.AluOpType

    nc = bacc.Bacc("TRN2", target_bir_lowering=False, debug=False,
                   num_devices=8)

    fpv_d = nc.dram_tensor("fpv", [2 * RA + 2 * RS, COLS], F16,
                           kind="ExternalInput").ap()
    wall_d = nc.dram_tensor("wall", [128, 448], F16,
                            kind="ExternalInput").ap()
    bias_d = nc.dram_tensor("bias", [128, 3], F32, kind="ExternalInput").ap()
    y_d = nc.dram_tensor("y", [128, YCOLS], F16, kind="ExternalOutput").ap()

    with TileContext(nc) as tc:
        with (tc.tile_pool(name="w", bufs=1) as wp,
              tc.tile_pool(name="fp", bufs=1) as fpool,
              tc.tile_pool(name="io", bufs=12) as io,
              tc.tile_pool(name="ps0", bufs=2, space="PSUM") as ps0,
              tc.tile_pool(name="ps1", bufs=2, space="PSUM") as ps1,
              tc.tile_pool(name="psy", bufs=2, space="PSUM") as psy):
            wall = wp.tile([128, 448], F16)
            biasw = wp.tile([128, 3], F32)
            dummy = wp.tile([128, NT], F16)
            fat = [fpool.tile([2 * RA, c1 - c0], F16, name=f"fa{i}")
                   for i, (c0, c1) in enumerate(FA_CHUNKS)]
            fst = [fpool.tile([2 * RS, c1 - c0], F16, name=f"fs{i}")
                   for i, (c0, c1) in enumerate(FS_CHUNKS)]

            def fsrc(chunks, tiles, c0, w):
                for (ck0, ck1), t in zip(chunks, tiles):
                    if ck0 <= c0 and c0 + w <= ck1:
                        return t[:, c0 - ck0:c0 - ck0 + w]
                raise AssertionError(f"no chunk covers {c0}+{w}")

            # DMA rings: SP = first group's chunk + outputs; Scalar
            # hwdge = weights/bias; Pool software ring = the remaining
            # bulk, dispatched in strict consumption order (DMA engines
            # are shared across rings, so service order ~ dispatch
            # order; out-of-order bulk starves the early tiles).
            nc.vector.memset(dummy[:], 0.0)
            nc.sync.dma_start(out=wall[:], in_=wall_d[:, :])
            nc.sync.dma_start(out=biasw[:], in_=bias_d[:, :])
            for i, (c0, c1) in enumerate(FA_CHUNKS):
                nc.gpsimd.dma_start(out=fat[i][:], in_=fpv_d[0:2 * RA, c0:c1])
            for i, (c0, c1) in enumerate(FS_CHUNKS):
                eng = nc.scalar if FS_RINGS[i] == "scalar" else nc.sync
                eng.dma_start(out=fst[i][:],
                              in_=fpv_d[2 * RA:2 * RA + 2 * RS, c0:c1])

            W0at = wall[0:2 * RA, 0:128]
            W0st = wall[0:2 * RS, 128:256]
            W1t = wall[0:128, 256:384]
            W2t = wall[0:128, 384:448]
            b0t = biasw[0:128, 0:1]
            b1t = biasw[0:128, 1:2]
            b2t64 = biasw[0:64, 2:3]
            b2t = biasw[0:128, 2:3]

            # per-group state (group NGRP is the 128-col tail, 1 tile)
            H0P, H0T, H1P, H1T, YP, YT = {}, {}, {}, {}, {}, {}

            def gw(g):
                return TNT if g == NGRP else NT

            def gtiles(g):
                return 1 if g == NGRP else 2

            def issue_W0(g, warm=0, fill=0):
                w = gw(g)
                n = gtiles(g)
                H0P[g] = ps0.tile([128, n * w], F32, name="h0p")
                for d in range(warm):
                    nc.tensor.matmul(out=H0P[g][:, 0:w - d],
                                     lhsT=dummy[:, 0:128],
                                     rhs=dummy[:, 0:w - d],
                                     start=(d == 0), stop=False)
                for t in range(n):
                    c0 = 2 * g * NT + t * NT if g < NGRP else NFULL * NT
                    nc.tensor.matmul(out=H0P[g][:, t * w:(t + 1) * w],
                                     lhsT=W0at,
                                     rhs=fsrc(FA_CHUNKS, fat, c0, w),
                                     start=(warm == 0 or t > 0), stop=False)
                # zero-weight fillers absorb input-supply jitter without
                # letting the PE idle (idle in the governor window demotes
                # the clock for the whole kernel)
                for d in range(fill):
                    nc.tensor.matmul(out=H0P[g][:, 0:w - 1 - d],
                                     lhsT=dummy[:, 0:128],
                                     rhs=dummy[:, 0:w - 1 - d],
                                     start=False, stop=False)
                for t in range(n):
                    c0 = 2 * g * NT + t * NT if g < NGRP else NFULL * NT
                    nc.tensor.matmul(out=H0P[g][:, t * w:(t + 1) * w],
                                     lhsT=W0st,
                                     rhs=fsrc(FS_CHUNKS, fst, c0, w),
                                     start=False, stop=True)

            def issue_relu0(g):
                w = gw(g) * gtiles(g)
                H0T[g] = io.tile([128, w], F16, name="h0t")
                nc.scalar.activation(H0T[g][:], H0P[g][:],
                                     AF.Relu, bias=b0t, scale=1.0)

            def issue_W1(g):
                w = gw(g)
                n = gtiles(g)
                H1P[g] = [ps1.tile([128, w], F32, name="h1p")
                          for _ in range(n)]
                for t in range(n):
                    nc.tensor.matmul(out=H1P[g][t][:], lhsT=W1t,
                                     rhs=H0T[g][:, t * w:(t + 1) * w],
                                     start=True, stop=True)

            def issue_relu1(g):
                w = gw(g)
                H1T[g] = [io.tile([128, w], F16, name="h1t")
                          for _ in range(gtiles(g))]
                for t in range(gtiles(g)):
                    nc.vector.tensor_scalar(out=H1T[g][t][:],
                                            in0=H1P[g][t][:],
                                            scalar1=b1t, scalar2=0.0,
                                            op0=ALU.add, op1=ALU.max)

            def issue_W2(g):
                w = gw(g)
                if g == NGRP:
                    YP[g] = psy.tile([64, w], F32, name="yp")
                    nc.tensor.matmul(out=YP[g][:], lhsT=W2t,
                                     rhs=H1T[g][0][:], start=True, stop=True)
                else:
                    YP[g] = psy.tile([128, w], F32, name="yp")
                    nc.tensor.matmul(out=YP[g][0:64, :], lhsT=W2t,
                                     rhs=H1T[g][0][:],
                                     start=True, stop=True)
                    nc.tensor.matmul(out=YP[g][64:128, :], lhsT=W2t,
                                     rhs=H1T[g][1][:], start=True,
                                     stop=True, tile_position=(0, 64))

            def issue_y(g):
                w = gw(g)
                if g == NGRP:
                    YT[g] = io.tile([64, w], F16, name="yt")
                    nc.scalar.activation(YT[g][:], YP[g][:], AF.Identity,
                                         bias=b2t64, scale=1.0)
                elif g % 2 == 0:
                    YT[g] = io.tile([128, w], F16, name="yt")
                    nc.scalar.activation(YT[g][:], YP[g][:], AF.Identity,
                                         bias=b2t, scale=1.0)
                else:
                    YT[g] = io.tile([128, w], F16, name="yt")
                    nc.vector.tensor_scalar(out=YT[g][:], in0=YP[g][:],
                                            scalar1=b2t, scalar2=None,
                                            op0=ALU.add)

            def issue_dma(g):
                # outputs split across the SP ring (idle after wall/
                # bias) and the Pool ring (idle after the fa chunks) so
                # out entries do not serialize behind one queue
                if g == NGRP:
                    nc.sync.dma_start(out=y_d[0:64, NGRP * NT:YCOLS],
                                      in_=YT[g][:])
                elif g == NGRP - 1:
                    # final group's output splits across SP + Scalar so
                    # the last (critical-path) transfer halves; Scalar's
                    # compute is already done by then, so its dispatch
                    # cost is free (it is NOT free mid-stream)
                    nc.sync.dma_start(
                        out=y_d[0:64, g * NT:(g + 1) * NT],
                        in_=YT[g][0:64, :])
                    nc.scalar.dma_start(
                        out=y_d[64:128, g * NT:(g + 1) * NT],
                        in_=YT[g][64:128, :])
                else:
                    nc.sync.dma_start(out=y_d[:, g * NT:(g + 1) * NT],
                                      in_=YT[g][:])

            # software-pipelined schedule; the 128-col tail group runs
            # SECOND so its serial mini-chain hides inside the early
            # pipeline and the kernel drains on a full group
            seq = [0, NGRP] + list(range(1, NGRP))
            issue_W0(seq[0], warm=16)
            issue_relu0(seq[0])
            for i, g in enumerate(seq):
                if i + 1 < len(seq):
                    issue_W0(seq[i + 1], fill=(1 if i < 2 else 0))
                    issue_relu0(seq[i + 1])
                issue_W1(g)
                issue_relu1(g)
                if i - 1 >= 0:
                    issue_W2(seq[i - 1])
                    issue_y(seq[i - 1])
                    issue_dma(seq[i - 1])
            issue_W2(seq[-1])
            issue_y(seq[-1])
            issue_dma(seq[-1])

    nc.compile()
    return nc


def _get_program():
    global _PREP
    if _PREP is None:
        _PREP = _build_program()
    return _PREP


def _pack_weights(W0, b0, W1, b1, W2, b2):
    """Per-expert [128, 448] fp16 weight wall + [128, 3] f32 biases."""
    W0a = W0[_W0A_ROWS].astype(np.float32)          # [38, 64]
    W0s = W0[_W0S_ROWS].astype(np.float32)          # [36, 64]
    wall = np.zeros((128, 448), np.float16)
    wall[0:RA, 0:64] = W0a
    wall[RA:2 * RA, 64:128] = W0a
    wall[0:RS, 128:192] = W0s
    wall[RS:2 * RS, 192:256] = W0s
    wall[0:64, 256:320] = W1
    wall[64:128, 320:384] = W1
    wall[0:64, 384:416] = W2
    wall[64:128, 416:448] = W2
    bias = np.zeros((128, 3), np.float32)
    bias[:, 0] = np.concatenate([b0, b0])
    bias[:, 1] = np.concatenate([b1, b1])
    bias[:, 2] = np.concatenate([b2, b2, b2, b2])
    return wall, bias


def _pack_cols(data, n):
    """[R, C-samples] -> [2R, COLS] pair-packed device layout."""
    R = data.shape[0]
    full = data[:, :NFULL * 2 * NT].reshape(R, NFULL, 2, NT)
    fullp = np.concatenate([full[:, :, 0], full[:, :, 1]],
                           axis=0).reshape(2 * R, NFULL * NT)
    tail = data[:, NFULL * 2 * NT:].reshape(R, 1, 2, TNT)
    tailp = np.concatenate([tail[:, :, 0], tail[:, :, 1]],
                           axis=0).reshape(2 * R, TNT)
    return np.concatenate([fullp, tailp], axis=1)


def _unpack_y(y):
    """[128, YCOLS] device layout -> [32, C] sample order.

    Full groups: y[0:64, g*NT:(g+1)*NT] is tile 2g, y[64:128, ...] is
    tile 2g+1; each [64, NT] tile holds samples [top 0:NT, bottom
    NT:2*NT].  Tail: y[0:64, NGRP*NT:] is the [64, TNT] tail tile."""
    parts = []
    for g in range(NGRP):
        blk = y[:, g * NT:(g + 1) * NT]
        for t64 in (blk[0:64], blk[64:128]):
            parts.append(np.concatenate([t64[0:32], t64[32:64]], axis=1))
    t64 = y[0:64, NGRP * NT:YCOLS]
    parts.append(np.concatenate([t64[0:32], t64[32:64]], axis=1))
    return np.concatenate(parts, axis=1)


def _s36(pos, view):
    """Posenc sin features in _W0S_ROWS order: pos sin (m=1,2), view sin
    (m=1,2,4,8), pos cos, view cos.  [n, 36] fp32."""
    sin_part = np.concatenate([pos, 2.0 * pos,
                               view, 2.0 * view, 4.0 * view, 8.0 * view],
                              axis=1).astype(np.float32)        # [n, 18]
    ang = np.concatenate([sin_part, sin_part + np.float32(0.5 * np.pi)],
                         axis=1)
    return np.sin(ang)


def kernel(idxs, positions, viewdirs, features, W0, b0, W1, b1, W2, b2):
    from concourse.bass_utils import run_bass_kernel_spmd

    N = idxs.shape[0]
    idx = idxs.reshape(-1).astype(np.int64)
    out = np.zeros((N, D), np.float32)

    # Route: list of (expert, sample-index-array) chunks of <= C samples.
    chunks = []
    for k in range(K):
        sel = np.nonzero(idx == k)[0]
        for lo in range(0, len(sel), C):
            chunks.append((k, sel[lo:lo + C]))

    walls = [_pack_weights(W0[k], b0[k], W1[k], b1[k], W2[k], b2[k])
             for k in range(K)]

    nc = _get_program()
    zero_in = None
    for inv in range(0, len(chunks), 8):
        batch = chunks[inv:inv + 8]
        in_maps = []
        for ci in range(8):
            if ci < len(batch):
                k, sel = batch[ci]
                n = len(sel)
                fpv = np.zeros((RA + RS, C), np.float16)
                fpv[0:32, :n] = features[sel].T
                fpv[32:35, :n] = positions[sel].T
                fpv[35:38, :n] = viewdirs[sel].T
                fpv[RA:RA + RS, :n] = _s36(positions[sel],
                                           viewdirs[sel]).T
                fa = _pack_cols(fpv[0:RA], n)          # [76, COLS]
                fs = _pack_cols(fpv[RA:RA + RS], n)    # [72, COLS]
                in_maps.append({"fpv": np.ascontiguousarray(
                                    np.concatenate([fa, fs], axis=0)),
                                "wall": walls[k][0],
                                "bias": walls[k][1]})
            else:
                if zero_in is None:
                    zero_in = {"fpv": np.zeros((2 * RA + 2 * RS, COLS),
                                               np.float16),
                               "wall": walls[0][0],
                               "bias": walls[0][1]}
                in_maps.append(zero_in)
        global _LAST_IN_MAPS
        _LAST_IN_MAPS = in_maps
        res = None
        for attempt in range(3):
            try:
                res = run_bass_kernel_spmd(nc, in_maps,
                                           core_ids=list(range(8)))
                break
            except Exception:
                if attempt == 2:
                    raise
        assert res is not None
        for ci, (k, sel) in enumerate(batch):
            y128 = np.asarray(res.results[ci]["y"], np.float32)
            y32 = _unpack_y(y128)
            out[sel] = y32[:, :len(sel)].T
    return out


# revision 61
# speedup vs baseline: 1.0077x; 1.0077x over previous
"""MoE post-processing MLP kernel for Trainium2 (8 NeuronCores).

Strategy: expert-parallel sharding. Each core is assigned one chunk of
samples routed to a single expert (K=8 experts ~= 8 cores for uniform
routing). The host gathers/permutes samples by expert, computes the 36
posenc sin features (np.sin is cheap host-side and more precise than
the fp16 device path), and the device runs a dense 3-layer MLP in fp16
(fp32 PSUM accumulation):

  h0 = relu(W0a^T@xa + W0s^T@s36 + b0)   xa = [feat,pos,view] 38 rows
  h1 = relu(W1^T@h0 + b1)                s36 = posenc sins, 36 rows
  y  = W2^T@h1 + b2

Device layout: pair-packed (2 samples per column; weights duplicated
block-diagonally so the full 128-partition contract dim is used).
COLS = 8*512 + 128 = 4224 columns = 8448 samples per core, sized to
the actual max per-expert count (8336) instead of a generic bound.

Processing unit is a 1024-col group (two 512-col PSUM tiles), with the
PE issue stream SOFTWARE-PIPELINED so every matmul's producers ran at
least one group earlier (no within-group PE->Scalar->PE round trips):
  iter i issues:  W0a/W0s(seq[i+1]) x4 | W1(seq[i]) x2 | W2(seq[i-1]) x2
where seq = [0, tail, 1, 2, 3]: the 128-col tail group runs second so
its serial mini-chain hides inside the early pipeline and the kernel
drains on a full group.
W2(t0) targets PSUM partitions 0:64 and W2(t1) partitions 64:128 of
ONE [128,512] bank via PE tile_position=(0,64) (the second tile of the
pair runs in the free half of the PE array at near-zero cost), so a
single full-width tensor_scalar emits y for both tiles (the ACT/DVE
fixed ~350-cycle per-op overhead makes many small ops the enemy).
Element-wise: relu0 per 1024 on Scalar (one activation Relu + b0 per
group), relu1 per 512 on Vector (add-b1/max-0, separate h1t tiles so
each W2 half starts as soon as its half is ready), y_pair alternating
Scalar/Vector per group; Pool cannot read PSUM on TRN2.
PSUM: h0p 1024x2bufs (4 banks) + h1p 512x2bufs (2) + yp 512x2bufs (2).

Input chunks are SEPARATE SBUF tiles (fa0/fa1/.., fs0/..): the Tile
framework tracks dependencies per tile, so a single [76, COLS] tile
would make the first matmul wait on the LAST bulk DMA chunk (observed
8us PE stall).  Rings: Pool software ring streams the fa chunks,
Scalar hwdge the fs chunks (per-queue transfers serialize at
~40-80 GB/s, so the two tensors ride separate queues in consumption
order); SP carries wall/bias and all y outputs.

The PE clock governor samples duty EARLY in the kernel (~8-14us) and
the decision sticks: any multi-us PE idle in that window locks the
kernel at 1.2 GHz (427ns/512-row matmul) while 100% duty yields
2.4 GHz (217ns).  Input DMA signals per-entry (first chunk ready ~10.5-12us), but the
combined queue supply (~150 GB/s) only just sustains full-rate
consumption, so the stream starts once ~2 groups of backlog exist: a
16-matmul zero-weight warmup chain (accumulating zeros into g0's live
h0p group, which survives DCE) bridges engine start to that point, and
one zero-weight filler matmul inside the first two lookahead groups'
open h0p accumulation absorbs supply jitter without letting the PE
idle; fillers past the ~16us governor window are pure cost.  The final
group's output splits into parallel SP + Scalar entries (Scalar's
compute is done by then; mid-stream Scalar dispatches steal from the
relu queue, and outputs must never ride the Pool software ring, whose
end-of-ring DRAIN would trail the last transfer by ~2us).
"""

import numpy as np

K = 8
WID = 64
D = 32
NT = 512            # full-tile matmul moving dim (one fp32 PSUM bank)
NFULL = 8           # full tiles (1024 samples each, pair-packed)
TNT = 128           # tail-tile moving dim (256 samples)
C = NFULL * 2 * NT + 2 * TNT     # 8448 samples per core-chunk
COLS = NFULL * NT + TNT          # 4224 device columns
NGRP = NFULL // 2   # 4 full groups of 1024 cols; group NGRP = tail
YCOLS = NGRP * NT + TNT          # 2176 output dram columns

RA = 38             # xa rows: feat 32 + pos 3 + view 3
RS = 36             # s36 rows

# input chunk boundaries (512-aligned; separate SBUF tiles per chunk).
# DMA packet length = chunk width * 2B per partition row; wider chunks
# amortize the ~20-45ns per-packet queue issue cost, but the first
# chunk must stay small so the pipeline can start early.  Queue rates
# measured: Pool swdge ~81 GB/s, Scalar hwdge ~42, SP hwdge ~30; fa
# rides Pool alone, fs alternates Scalar/SP so three queues stream in
# parallel, each in consumption order.
FA_CHUNKS = [(0, 512), (512, 1024), (4096, COLS), (1024, 2048),
             (2048, 3072), (3072, 4096)]
FS_CHUNKS = [(0, 512), (512, 1024), (4096, COLS), (1024, 2048),
             (2048, 3072), (3072, 4096)]
FS_RINGS = ["scalar"] * 6

# W0 row indices (DIN=74 layout: feat 0:32, posenc(pos,2) 32:47,
# posenc(view,4) 47:74) for the identity part and the sin part.
_W0A_ROWS = list(range(32)) + [32, 33, 34] + [47, 48, 49]
_W0S_ROWS = (list(range(35, 41)) + list(range(50, 62))
             + list(range(41, 47)) + list(range(62, 74)))

_PREP = None  # compiled Bass program, built once per process
_LAST_IN_MAPS = None  # stashed for external profiling harnesses


def _build_program():
    import concourse.bacc as bacc
    import concourse.mybir as mybir
    from concourse.tile import TileContext

    F32, F16 = mybir.dt.float32, mybir.dt.float16
    AF = mybir.ActivationFunctionType
    ALU = mybir.AluOpType

    nc = bacc.Bacc("TRN2", target_bir_lowering=False, debug=False,
                   num_devices=8)

    fpv_d = nc.dram_tensor("fpv", [2 * RA + 2 * RS, COLS], F16,
                           kind="ExternalInput").ap()
    wall_d = nc.dram_tensor("wall", [128, 448], F16,
                            kind="ExternalInput").ap()
    bias_d = nc.dram_tensor("bias", [128, 3], F32, kind="ExternalInput").ap()
    y_d = nc.dram_tensor("y", [128, YCOLS], F16, kind="ExternalOutput").ap()

    with TileContext(nc) as tc:
        with (tc.tile_pool(name="w", bufs=1) as wp,
              tc.tile_pool(name="fp", bufs=1) as fpool,
              tc.tile_pool(name="io", bufs=12) as io,
              tc.tile_pool(name="ps0", bufs=2, space="PSUM") as ps0,
              tc.tile_pool(name="ps1", bufs=2, space="PSUM") as ps1,
              tc.tile_pool(name="psy", bufs=2, space="PSUM") as psy):
            wall = wp.tile([128, 448], F16)
            biasw = wp.tile([128, 3], F32)
            dummy = wp.tile([128, NT], F16)
            fat = [fpool.tile([2 * RA, c1 - c0], F16, name=f"fa{i}")
                   for i, (c0, c1) in enumerate(FA_CHUNKS)]
            fst = [fpool.tile([2 * RS, c1 - c0], F16, name=f"fs{i}")
                   for i, (c0, c1) in enumerate(FS_CHUNKS)]

            def fsrc(chunks, tiles, c0, w):
                for (ck0, ck1), t in zip(chunks, tiles):
                    if ck0 <= c0 and c0 + w <= ck1:
                        return t[:, c0 - ck0:c0 - ck0 + w]
                raise AssertionError(f"no chunk covers {c0}+{w}")

            # DMA rings: SP = first group's chunk + outputs; Scalar
            # hwdge = weights/bias; Pool software ring = the remaining
            # bulk, dispatched in strict consumption order (DMA engines
            # are shared across rings, so service order ~ dispatch
            # order; out-of-order bulk starves the early tiles).
            nc.vector.memset(dummy[:], 0.0)
            nc.sync.dma_start(out=wall[:], in_=wall_d[:, :])
            nc.sync.dma_start(out=biasw[:], in_=bias_d[:, :])
            for i, (c0, c1) in enumerate(FA_CHUNKS):
                nc.gpsimd.dma_start(out=fat[i][:], in_=fpv_d[0:2 * RA, c0:c1])
            for i, (c0, c1) in enumerate(FS_CHUNKS):
                eng = nc.scalar if FS_RINGS[i] == "scalar" else nc.sync
                eng.dma_start(out=fst[i][:],
                              in_=fpv_d[2 * RA:2 * RA + 2 * RS, c0:c1])

            W0at = wall[0:2 * RA, 0:128]
            W0st = wall[0:2 * RS, 128:256]
            W1t = wall[0:128, 256:384]
            W2t = wall[0:128, 384:448]
            b0t = biasw[0:128, 0:1]
            b1t = biasw[0:128, 1:2]
            b2t64 = biasw[0:64, 2:3]
            b2t = biasw[0:128, 2:3]

            # per-group state (group NGRP is the 128-col tail, 1 tile)
            H0P, H0T, H1P, H1T, YP, YT = {}, {}, {}, {}, {}, {}

            def gw(g):
                return TNT if g == NGRP else NT

            def gtiles(g):
                return 1 if g == NGRP else 2

            def issue_W0(g, warm=0, fill=0):
                w = gw(g)
                n = gtiles(g)
                H0P[g] = ps0.tile([128, n * w], F32, name="h0p")
                for d in range(warm):
                    nc.tensor.matmul(out=H0P[g][:, 0:w - d],
                                     lhsT=dummy[:, 0:128],
                                     rhs=dummy[:, 0:w - d],
                                     start=(d == 0), stop=False)
                for t in range(n):
                    c0 = 2 * g * NT + t * NT if g < NGRP else NFULL * NT
                    nc.tensor.matmul(out=H0P[g][:, t * w:(t + 1) * w],
                                     lhsT=W0at,
                                     rhs=fsrc(FA_CHUNKS, fat, c0, w),
                                     start=(warm == 0 or t > 0), stop=False)
                # zero-weight fillers absorb input-supply jitter without
                # letting the PE idle (idle in the governor window demotes
                # the clock for the whole kernel)
                for d in range(fill):
                    nc.tensor.matmul(out=H0P[g][:, 0:w - 1 - d],
                                     lhsT=dummy[:, 0:128],
                                     rhs=dummy[:, 0:w - 1 - d],
                                     start=False, stop=False)
                for t in range(n):
                    c0 = 2 * g * NT + t * NT if g < NGRP else NFULL * NT
                    nc.tensor.matmul(out=H0P[g][:, t * w:(t + 1) * w],
                                     lhsT=W0st,
                                     rhs=fsrc(FS_CHUNKS, fst, c0, w),
                                     start=False, stop=True)

            def issue_relu0(g, split=False):
                w = gw(g)
                n = gtiles(g)
                if split:
                    # per-512 halves in separate tiles: deps are
                    # tile-granular, so W1(t0) starts ~0.9us earlier —
                    # critical only for g0 (the prologue has no prior
                    # work to hide the full-width relu0 latency behind)
                    H0T[g] = [io.tile([128, w], F16, name="h0t")
                              for _ in range(n)]
                    for t in range(n):
                        nc.scalar.activation(
                            H0T[g][t][:], H0P[g][:, t * w:(t + 1) * w],
                            AF.Relu, bias=b0t, scale=1.0)
                else:
                    H0T[g] = io.tile([128, n * w], F16, name="h0t")
                    nc.scalar.activation(H0T[g][:], H0P[g][:],
                                         AF.Relu, bias=b0t, scale=1.0)

            def issue_W1(g):
                w = gw(g)
                n = gtiles(g)
                H1P[g] = [ps1.tile([128, w], F32, name="h1p")
                          for _ in range(n)]
                for t in range(n):
                    rhs = (H0T[g][t][:] if isinstance(H0T[g], list)
                           else H0T[g][:, t * w:(t + 1) * w])
                    nc.tensor.matmul(out=H1P[g][t][:], lhsT=W1t,
                                     rhs=rhs, start=True, stop=True)

            def issue_relu1(g):
                w = gw(g)
                H1T[g] = [io.tile([128, w], F16, name="h1t")
                          for _ in range(gtiles(g))]
                for t in range(gtiles(g)):
                    nc.vector.tensor_scalar(out=H1T[g][t][:],
                                            in0=H1P[g][t][:],
                                            scalar1=b1t, scalar2=0.0,
                                            op0=ALU.add, op1=ALU.max)

            def issue_W2(g):
                w = gw(g)
                if g == NGRP:
                    YP[g] = psy.tile([64, w], F32, name="yp")
                    nc.tensor.matmul(out=YP[g][:], lhsT=W2t,
                                     rhs=H1T[g][0][:], start=True, stop=True)
                else:
                    YP[g] = psy.tile([128, w], F32, name="yp")
                    nc.tensor.matmul(out=YP[g][0:64, :], lhsT=W2t,
                                     rhs=H1T[g][0][:],
                                     start=True, stop=True)
                    nc.tensor.matmul(out=YP[g][64:128, :], lhsT=W2t,
                                     rhs=H1T[g][1][:], start=True,
                                     stop=True, tile_position=(0, 64))

            def issue_y(g):
                w = gw(g)
                if g == NGRP:
                    YT[g] = io.tile([64, w], F16, name="yt")
                    nc.scalar.activation(YT[g][:], YP[g][:], AF.Identity,
                                         bias=b2t64, scale=1.0)
                elif g % 2 == 0:
                    YT[g] = io.tile([128, w], F16, name="yt")
                    nc.scalar.activation(YT[g][:], YP[g][:], AF.Identity,
                                         bias=b2t, scale=1.0)
                else:
                    YT[g] = io.tile([128, w], F16, name="yt")
                    nc.vector.tensor_scalar(out=YT[g][:], in0=YP[g][:],
                                            scalar1=b2t, scalar2=None,
                                            op0=ALU.add)

            def issue_dma(g):
                # outputs split across the SP ring (idle after wall/
                # bias) and the Pool ring (idle after the fa chunks) so
                # out entries do not serialize behind one queue
                if g == NGRP:
                    nc.sync.dma_start(out=y_d[0:64, NGRP * NT:YCOLS],
                                      in_=YT[g][:])
                elif g == NGRP - 1:
                    # final group's output splits across SP + Scalar so
                    # the last (critical-path) transfer halves; Scalar's
                    # compute is already done by then, so its dispatch
                    # cost is free (it is NOT free mid-stream)
                    nc.sync.dma_start(
                        out=y_d[0:64, g * NT:(g + 1) * NT],
                        in_=YT[g][0:64, :])
                    nc.scalar.dma_start(
                        out=y_d[64:128, g * NT:(g + 1) * NT],
                        in_=YT[g][64:128, :])
                else:
                    nc.sync.dma_start(out=y_d[:, g * NT:(g + 1) * NT],
                                      in_=YT[g][:])

            # software-pipelined schedule; the 128-col tail group runs
            # SECOND so its serial mini-chain hides inside the early
            # pipeline and the kernel drains on a full group
            seq = [0, NGRP] + list(range(1, NGRP))
            issue_W0(seq[0], warm=16)
            issue_relu0(seq[0], split=True)
            for i, g in enumerate(seq):
                if i + 1 < len(seq):
                    issue_W0(seq[i + 1], fill=(1 if i < 2 else 0))
                    issue_relu0(seq[i + 1])
                issue_W1(g)
                issue_relu1(g)
                if i - 1 >= 0:
                    issue_W2(seq[i - 1])
                    issue_y(seq[i - 1])
                    issue_dma(seq[i - 1])
            issue_W2(seq[-1])
            issue_y(seq[-1])
            issue_dma(seq[-1])

    nc.compile()
    return nc


def _get_program():
    global _PREP
    if _PREP is None:
        _PREP = _build_program()
    return _PREP


def _pack_weights(W0, b0, W1, b1, W2, b2):
    """Per-expert [128, 448] fp16 weight wall + [128, 3] f32 biases."""
    W0a = W0[_W0A_ROWS].astype(np.float32)          # [38, 64]
    W0s = W0[_W0S_ROWS].astype(np.float32)          # [36, 64]
    wall = np.zeros((128, 448), np.float16)
    wall[0:RA, 0:64] = W0a
    wall[RA:2 * RA, 64:128] = W0a
    wall[0:RS, 128:192] = W0s
    wall[RS:2 * RS, 192:256] = W0s
    wall[0:64, 256:320] = W1
    wall[64:128, 320:384] = W1
    wall[0:64, 384:416] = W2
    wall[64:128, 416:448] = W2
    bias = np.zeros((128, 3), np.float32)
    bias[:, 0] = np.concatenate([b0, b0])
    bias[:, 1] = np.concatenate([b1, b1])
    bias[:, 2] = np.concatenate([b2, b2, b2, b2])
    return wall, bias


def _pack_cols(data, n):
    """[R, C-samples] -> [2R, COLS] pair-packed device layout."""
    R = data.shape[0]
    full = data[:, :NFULL * 2 * NT].reshape(R, NFULL, 2, NT)
    fullp = np.concatenate([full[:, :, 0], full[:, :, 1]],
                           axis=0).reshape(2 * R, NFULL * NT)
    tail = data[:, NFULL * 2 * NT:].reshape(R, 1, 2, TNT)
    tailp = np.concatenate([tail[:, :, 0], tail[:, :, 1]],
                           axis=0).reshape(2 * R, TNT)
    return np.concatenate([fullp, tailp], axis=1)


def _unpack_y(y):
    """[128, YCOLS] device layout -> [32, C] sample order.

    Full groups: y[0:64, g*NT:(g+1)*NT] is tile 2g, y[64:128, ...] is
    tile 2g+1; each [64, NT] tile holds samples [top 0:NT, bottom
    NT:2*NT].  Tail: y[0:64, NGRP*NT:] is the [64, TNT] tail tile."""
    parts = []
    for g in range(NGRP):
        blk = y[:, g * NT:(g + 1) * NT]
        for t64 in (blk[0:64], blk[64:128]):
            parts.append(np.concatenate([t64[0:32], t64[32:64]], axis=1))
    t64 = y[0:64, NGRP * NT:YCOLS]
    parts.append(np.concatenate([t64[0:32], t64[32:64]], axis=1))
    return np.concatenate(parts, axis=1)


def _s36(pos, view):
    """Posenc sin features in _W0S_ROWS order: pos sin (m=1,2), view sin
    (m=1,2,4,8), pos cos, view cos.  [n, 36] fp32."""
    sin_part = np.concatenate([pos, 2.0 * pos,
                               view, 2.0 * view, 4.0 * view, 8.0 * view],
                              axis=1).astype(np.float32)        # [n, 18]
    ang = np.concatenate([sin_part, sin_part + np.float32(0.5 * np.pi)],
                         axis=1)
    return np.sin(ang)


def kernel(idxs, positions, viewdirs, features, W0, b0, W1, b1, W2, b2):
    from concourse.bass_utils import run_bass_kernel_spmd

    N = idxs.shape[0]
    idx = idxs.reshape(-1).astype(np.int64)
    out = np.zeros((N, D), np.float32)

    # Route: list of (expert, sample-index-array) chunks of <= C samples.
    chunks = []
    for k in range(K):
        sel = np.nonzero(idx == k)[0]
        for lo in range(0, len(sel), C):
            chunks.append((k, sel[lo:lo + C]))

    walls = [_pack_weights(W0[k], b0[k], W1[k], b1[k], W2[k], b2[k])
             for k in range(K)]

    nc = _get_program()
    zero_in = None
    for inv in range(0, len(chunks), 8):
        batch = chunks[inv:inv + 8]
        in_maps = []
        for ci in range(8):
            if ci < len(batch):
                k, sel = batch[ci]
                n = len(sel)
                fpv = np.zeros((RA + RS, C), np.float16)
                fpv[0:32, :n] = features[sel].T
                fpv[32:35, :n] = positions[sel].T
                fpv[35:38, :n] = viewdirs[sel].T
                fpv[RA:RA + RS, :n] = _s36(positions[sel],
                                           viewdirs[sel]).T
                fa = _pack_cols(fpv[0:RA], n)          # [76, COLS]
                fs = _pack_cols(fpv[RA:RA + RS], n)    # [72, COLS]
                in_maps.append({"fpv": np.ascontiguousarray(
                                    np.concatenate([fa, fs], axis=0)),
                                "wall": walls[k][0],
                                "bias": walls[k][1]})
            else:
                if zero_in is None:
                    zero_in = {"fpv": np.zeros((2 * RA + 2 * RS, COLS),
                                               np.float16),
                               "wall": walls[0][0],
                               "bias": walls[0][1]}
                in_maps.append(zero_in)
        global _LAST_IN_MAPS
        _LAST_IN_MAPS = in_maps
        res = None
        for attempt in range(3):
            try:
                res = run_bass_kernel_spmd(nc, in_maps,
                                           core_ids=list(range(8)))
                break
            except Exception:
                if attempt == 2:
                    raise
        assert res is not None
        for ci, (k, sel) in enumerate(batch):
            y128 = np.asarray(res.results[ci]["y"], np.float32)
            y32 = _unpack_y(y128)
            out[sel] = y32[:, :len(sel)].T
    return out
